# revision 2
# baseline (speedup 1.0000x reference)
"""Trainium2 Bass kernel for nn_MultiHeadAttention_46093589021334.

Transformer-XL style multi-head attention with SCALE = 1/D**5 ~= 9.3e-10
(faithful to the source module). At that scale every attention logit is
O(1e-9) after scaling, so softmax(attn * SCALE) equals the uniform
distribution over unmasked key positions to one part in 1e8 -- far below
fp32 roundoff of the reference itself.  The module output is therefore
(exactly, to fp32 precision):

    out[:, b, :] = (M @ emb_b) @ Wkv[:, H*D:] @ Wfc

where emb_b = concat(emb_old, emb_new)[:, b, :]  (klen x emb) and
M[t, j] = (not mask[t, j]) / (# unmasked j in row t)   (q x klen).

Two algebraic restructurings on top of the baseline:

1. Weight folding (host, compile-time): W2 = Wkv[:, H*D:] @ Wfc is a
   fixed [emb, emb] matrix -- fold it once on the host.  The device chain
   becomes two matmuls instead of three.

2. Prefix structure of M: row t of the unnormalized not-mask sums ALL of
   emb_old plus a PREFIX of emb_new (j <= t).  So on device:
       A'.T[e, t] = colsum(emb_old)[e] + sum_{j<=t} emb_new[j, e]
   The prefix term is a matmul against a [512, 512] lower-triangular
   ones matrix L (generated on device by iota+is_ge, block-clipped:
   only 10240 PE cycles instead of 27648 for the dense masked matmul).
   The colsum term is a DVE free-dim reduction over a host-transposed
   copy of emb_old, folded in for free as a per-partition bias during
   the PSUM->SBUF evacuation of the prefix result.
   The 1/count(t) row normalization is diagonal on the t axis (the
   moving/free axis of both matmuls) so it commutes to the very end and
   is applied on the host during the gather, as in the baseline.

Everything on device runs in fp16 (tolerance is 2e-2; fp16 adds ~1e-3):
halves DMA traffic and enables fast weight load on the PE.

Distribution: data-parallel over batch. BATCH == 8 == n_cores; no
collectives.  Outputs are produced transposed ([emb, q]) so the chain
needs no on-device transposes.
"""

import sys

if "/opt/trn_rl_repo" not in sys.path:
    sys.path.insert(0, "/opt/trn_rl_repo")

import numpy as np

P = 128
Q_LEN = 512
MEM_LEN = 512
KLEN = 1024
BATCH = 8
EMB = 1024
HD = 1024  # H * D
N_CORES = 8
NE = EMB // P    # e tiles (8)
NKN = Q_LEN // P  # new-key tiles (4)

_PROGRAM_CACHE = {}


def _build_program():
    """Build + bacc-compile the per-core Bass program (cached)."""
    import concourse.bacc as bacc
    import concourse.mybir as mybir
    import concourse.tile as tile

    nc = bacc.Bacc(
        "TRN2",
        target_bir_lowering=False,
        debug=False,
        enable_asserts=False,
        num_devices=N_CORES,
    )
    f32 = mybir.dt.float32
    f16 = mybir.dt.float16

    emb_new = nc.dram_tensor("emb_new", [Q_LEN, EMB], f16, kind="ExternalInput").ap()
    emb_oldT = nc.dram_tensor("emb_oldT", [EMB, MEM_LEN], f16, kind="ExternalInput").ap()
    w2 = nc.dram_tensor("w2", [EMB, EMB], f16, kind="ExternalInput").ap()
    out_t = nc.dram_tensor("outT", [EMB, Q_LEN], f32, kind="ExternalOutput").ap()

    with tile.TileContext(nc) as tc:
        with (
            tc.tile_pool(name="sb", bufs=1) as sb,
            tc.tile_pool(name="ps", bufs=8, space="PSUM") as ps,
        ):
            sl = lambda m: slice(m * P, (m + 1) * P)

            # ---- device-generated lower-triangular ones blocks ----
            # lt[k][jj, t] = 1.0 iff (128k + jj) <= t.  iota val = t - jj.
            iota_t = sb.tile([P, Q_LEN], f32, tag="iota")
            nc.gpsimd.iota(
                iota_t[:], [[1, Q_LEN]], base=0, channel_multiplier=-1,
                allow_small_or_imprecise_dtypes=True,
            )
            lt = []
            for k in range(NKN):
                t = sb.tile([P, Q_LEN], f16, tag=f"lt{k}")
                nc.vector.tensor_scalar(
                    t[:], iota_t[:], float(k * P), None, mybir.AluOpType.is_ge
                )
                lt.append(t)

            # ---- input DMAs, all on the ACT HWDGE ring in consumption
            # order: emb_new (prefix matmuls) -> emb_oldT (colsum biases)
            # -> w2 (main matmul weights).  Outputs use the SP ring. ----
            en = []
            for k in range(NKN):
                t = sb.tile([P, EMB], f16, tag=f"en{k}")
                nc.scalar.dma_start(t[:], emb_new[k * P:(k + 1) * P, :])
                en.append(t)
            eo = []
            for m in range(NE):
                t = sb.tile([P, MEM_LEN], f16, tag=f"eo{m}")
                nc.scalar.dma_start(t[:], emb_oldT[m * P:(m + 1) * P, :])
                eo.append(t)
            w2t = []
            for e in range(NE):
                t = sb.tile([P, EMB], f16, tag=f"w2{e}")
                nc.scalar.dma_start(t[:], w2[e * P:(e + 1) * P, :])
                w2t.append(t)

            # ---- colsum(emb_old) on DVE: free-dim reduce of the
            # transposed old embeddings -> per-partition bias column ----
            cs = sb.tile([P, NE], f32, tag="cs")
            for m in range(NE):
                nc.vector.tensor_reduce(
                    cs[:, m:m + 1], eo[m][:],
                    mybir.AxisListType.X, mybir.AluOpType.add,
                )

            # ---- PE warmup: dummy matmuls keep the PE busy through the
            # HAM activity window while emb_new DMAs land ----
            warm = ps.tile([P, Q_LEN], f32, tag="psum", name="warm")
            for _ in range(5):
                nc.tensor.matmul(
                    warm[:], lhsT=lt[0][:, :P], rhs=lt[0][:],
                    start=True, stop=True,
                )

            # ---- phase 1 (k-outer): prefix-sum matmuls ----
            # psA[m][ee, t] = sum_{j<=t} emb_new[j, 128m+ee]
            # Block clipping: tile k's rhs is all-zero for t < 128k.
            psA = [
                ps.tile([P, Q_LEN], f32, tag="psum", name=f"psA{m}")
                for m in range(NE)
            ]
            for k in range(NKN):
                off = k * P
                for m in range(NE):
                    nc.tensor.matmul(
                        psA[m][:, off:] if k else psA[m][:],
                        lhsT=en[k][:, sl(m)],
                        rhs=lt[k][:, off:] if k else lt[k][:],
                        start=(k == 0),
                        stop=(k == NKN - 1),
                    )

            # ---- evacuate phase 1 with the colsum bias folded in, split
            # across ACT (m 0-3, biases arrive earliest) and DVE (m 4-7) ----
            at = []
            for m in range(NE):
                o = sb.tile([P, Q_LEN], f16, tag=f"at{m}")
                if m < 4:
                    nc.scalar.activation(
                        o[:], psA[m][:],
                        mybir.ActivationFunctionType.Identity,
                        bias=cs[:, m:m + 1],
                    )
                else:
                    nc.vector.tensor_scalar(
                        o[:], psA[m][:], cs[:, m:m + 1], None,
                        mybir.AluOpType.add,
                    )
                at.append(o)

            # ---- phase 2 (g-outer): outT'[g, t] = sum_e w2[e, g] A'.T[e, t]
            # g-outer staggers the 8 output tiles so copies + output DMA
            # overlap the remaining matmuls ----
            for g in range(NE):
                acc = ps.tile([P, Q_LEN], f32, tag="psum", name=f"psO{g}")
                for e in range(NE):
                    nc.tensor.matmul(
                        acc[:], lhsT=w2t[e][:, sl(g)], rhs=at[e][:],
                        start=(e == 0), stop=(e == NE - 1),
                    )
                o = sb.tile([P, Q_LEN], f32, tag=f"ot{g}")
                if g % 2 == 0:
                    nc.vector.tensor_copy(o[:], acc[:])
                else:
                    nc.scalar.copy(o[:], acc[:])
                nc.sync.dma_start(out_t[sl(g), :], o[:])

    nc.compile()
    return nc


def _get_program():
    if "nc" not in _PROGRAM_CACHE:
        _PROGRAM_CACHE["nc"] = _build_program()
    return _PROGRAM_CACHE["nc"]


def _make_in_maps(inputs):
    emb_new = np.asarray(inputs["emb_new"], dtype=np.float32)
    emb_old = np.asarray(inputs["emb_old"], dtype=np.float32)
    wkv = np.asarray(inputs["Wkv"], dtype=np.float32)
    wfc = np.asarray(inputs["Wfc"], dtype=np.float32)
    mask = np.asarray(inputs["mask"]).reshape(Q_LEN, KLEN)

    # 1/count row normalization (diagonal on t; commutes to the end).
    nm = ~mask
    inv_count = (1.0 / nm.sum(axis=1)).astype(np.float64)  # [q]

    # Compile-time weight folding: W2 = Wkv_v @ Wfc.
    w2 = (wkv[:, HD:].astype(np.float64) @ wfc.astype(np.float64)).astype(np.float16)

    in_maps = []
    for b in range(N_CORES):
        in_maps.append(
            {
                "emb_new": np.ascontiguousarray(emb_new[:, b, :]).astype(np.float16),
                "emb_oldT": np.ascontiguousarray(emb_old[:, b, :].T).astype(np.float16),
                "w2": w2,
            }
        )
    return in_maps, inv_count


def _run(inputs, trace=False, trace_cores=None):
    from concourse import bass_utils

    nc = _get_program()
    in_maps, inv_count = _make_in_maps(inputs)
    res = bass_utils.run_bass_kernel_spmd(
        nc,
        in_maps,
        core_ids=list(range(N_CORES)),
        trace=trace,
        trace_cores=trace_cores,
    )
    scale = inv_count[:, None].astype(np.float32)  # [q, 1]
    out = np.empty((Q_LEN, BATCH, EMB), dtype=np.float32)
    for b in range(N_CORES):
        out[:, b, :] = res.results[b]["outT"].T * scale
    return out, res


def _mask_is_causal(mask):
    qi = np.arange(Q_LEN)[:, None]
    ki = np.arange(KLEN)[None, :]
    return bool(np.array_equal(mask, ki > (qi + MEM_LEN)))


def _host_fallback(inputs, mask):
    """Numpy masked-mean path, used only if the mask is not the standard
    causal-with-memory pattern baked into the device program."""
    emb_new = np.asarray(inputs["emb_new"], dtype=np.float64)
    emb_old = np.asarray(inputs["emb_old"], dtype=np.float64)
    wkv = np.asarray(inputs["Wkv"], dtype=np.float64)
    wfc = np.asarray(inputs["Wfc"], dtype=np.float64)
    nm = (~mask).astype(np.float64)
    m = nm / nm.sum(axis=1, keepdims=True)
    emb_full = np.concatenate([emb_old, emb_new], axis=0)
    x = np.einsum("qk,kbe->qbe", m, emb_full)
    return (x @ wkv[:, HD:] @ wfc).astype(np.float32)


def kernel(**inputs):
    mask = np.asarray(inputs["mask"]).reshape(Q_LEN, KLEN)
    if not _mask_is_causal(mask):
        return _host_fallback(inputs, mask)
    out, _ = _run(inputs)
    return out


# revision 4
# speedup vs baseline: 1.3386x; 1.3386x over previous
"""Trainium2 Bass kernel for nn_MultiHeadAttention_46093589021334.

Transformer-XL style multi-head attention with SCALE = 1/D**5 ~= 9.3e-10
(faithful to the source module). At that scale every attention logit is
O(1e-9) after scaling, so softmax(attn * SCALE) equals the uniform
distribution over unmasked key positions to one part in 1e8 -- far below
fp32 roundoff of the reference itself.  The module output is therefore
(exactly, to fp32 precision):

    out[:, b, :] = (M @ emb_b) @ Wkv[:, H*D:] @ Wfc

where emb_b = concat(emb_old, emb_new)[:, b, :]  (klen x emb) and
M[t, j] = (not mask[t, j]) / (# unmasked j in row t)   (q x klen).

Restructurings on top of the baseline:

1. Weight folding (host, compile-time): W2 = Wkv[:, H*D:] @ Wfc is a
   fixed [emb, emb] matrix -- fold it once on the host.  The device
   chain becomes two matmuls instead of three.

2. Prefix structure of M: row t of the unnormalized not-mask sums ALL
   of emb_old plus a PREFIX of emb_new (j <= t).  On device:
       A'.T[e, t] = colsum(emb_old)[e] + sum_{j<=t} emb_new[j, e]
   The prefix is a matmul against a lower-triangular ones matrix; only
   the 128x128 diagonal block is ever triangular, so a single SBUF
   tile LW = [tri(128) | ones(512)] serves every k-wave as a prefix
   slice LW[:, 0:512-128k] (10240 PE cycles vs 27648 for the dense
   masked matmul).  The colsum term is input prep on the host (4 KB
   bias tensor, same spirit as the baseline's host-side inv_count),
   folded in for free as a per-partition bias during the PSUM->SBUF
   evacuation.  The 1/count(t) normalization is diagonal on t (the
   moving axis of both matmuls), so it commutes to the end and is
   applied on the host during the gather, as in the baseline.

Everything on device runs in fp16 (tolerance 2e-2; fp16 adds ~1e-3):
halves DMA traffic and enables fast weight load on the PE.

Scheduling: all DMAs ride the SP (sync) HWDGE ring in consumption
order (cs, emb_new, w2, outputs), keeping the ACT engine free for
PSUM evacuations (v2 lost ~10us to copies queued behind input DMAs).
The main matmul is split: an e-outer half consumes w2 tiles as they
arrive off DMA; a g-outer half staggers the output tiles so copies and
output DMA overlap the tail matmuls.  PSUM: 5 banks cycle the prefix
accumulators, 3 the main-phase accumulators.

Distribution: data-parallel over batch. BATCH == 8 == n_cores; no
collectives.  Outputs are produced transposed ([emb, q]); host
re-transposes during the gather.
"""

import sys

if "/opt/trn_rl_repo" not in sys.path:
    sys.path.insert(0, "/opt/trn_rl_repo")

import numpy as np

P = 128
Q_LEN = 512
MEM_LEN = 512
KLEN = 1024
BATCH = 8
EMB = 1024
HD = 1024  # H * D
N_CORES = 8
NE = EMB // P     # e tiles (8)
NKN = Q_LEN // P  # new-key tiles (4)

_PROGRAM_CACHE = {}


def _build_program():
    """Build + bacc-compile the per-core Bass program (cached)."""
    import concourse.bacc as bacc
    import concourse.mybir as mybir
    import concourse.tile as tile

    nc = bacc.Bacc(
        "TRN2",
        target_bir_lowering=False,
        debug=False,
        enable_asserts=False,
        num_devices=N_CORES,
    )
    f32 = mybir.dt.float32
    f16 = mybir.dt.float16

    emb_new = nc.dram_tensor("emb_new", [Q_LEN, EMB], f16, kind="ExternalInput").ap()
    cs_in = nc.dram_tensor("cs", [P, NE], f32, kind="ExternalInput").ap()
    w2 = nc.dram_tensor("w2", [EMB, EMB], f16, kind="ExternalInput").ap()
    out_t = nc.dram_tensor("outT", [EMB, Q_LEN], f16, kind="ExternalOutput").ap()

    with tile.TileContext(nc) as tc:
        with (
            tc.tile_pool(name="sb", bufs=1) as sb,
            tc.tile_pool(name="psA", bufs=5, space="PSUM") as psa_pool,
            tc.tile_pool(name="psO", bufs=3, space="PSUM") as pso_pool,
        ):
            sl = lambda m: slice(m * P, (m + 1) * P)

            # ---- LW = [tri(128) | ones(512)]: wave k's prefix operand is
            # LW[:, 0:512-128k].  ones via DVE memset; the triangular
            # diagonal block via gpsimd iota + DVE is_ge. ----
            LW = sb.tile([P, P + Q_LEN], f16, tag="LW")
            nc.vector.memset(LW[:], 1.0)
            iota_t = sb.tile([P, P], f32, tag="iota")
            nc.gpsimd.iota(
                iota_t[:], [[1, P]], base=0, channel_multiplier=-1,
                allow_small_or_imprecise_dtypes=True,
            )
            nc.vector.tensor_scalar(
                LW[:, 0:P], iota_t[:], 0.0, None, mybir.AluOpType.is_ge
            )

            # ---- input DMAs, all on the SP (sync) HWDGE ring in
            # consumption order ----
            cs = sb.tile([P, NE], f32, tag="cs")
            nc.sync.dma_start(cs[:], cs_in[:, :])
            en = []
            for k in range(NKN):
                t = sb.tile([P, EMB], f16, tag=f"en{k}")
                nc.sync.dma_start(t[:], emb_new[k * P:(k + 1) * P, :])
                en.append(t)
            w2t = []
            for e in range(NE):
                t = sb.tile([P, EMB], f16, tag=f"w2{e}")
                nc.sync.dma_start(t[:], w2[e * P:(e + 1) * P, :])
                w2t.append(t)

            # ---- PE warmup on the all-ones span of LW: keeps the PE busy
            # through the HAM activity window while DMAs land ----
            warm = pso_pool.tile([P, Q_LEN], f32, tag="psO", name="warm")
            for _ in range(4):
                nc.tensor.matmul(
                    warm[:], lhsT=LW[:, P:2 * P], rhs=LW[:, P:],
                    start=True, stop=True,
                )

            # ---- phase 1: prefix-sum matmuls ----
            # psA[m][ee, t] = sum_{j<=t} emb_new[j, 128m+ee]
            # wave k covers output columns [128k, 512): tri block at the
            # diagonal, ones beyond -- both are the prefix LW[:, :512-128k].
            psA = [psa_pool.tile([P, Q_LEN], f32, tag="psA", name=f"psA{m}") for m in range(NE)]

            def prefix_mm(m, k):
                nc.tensor.matmul(
                    psA[m][:, k * P:],
                    lhsT=en[k][:, sl(m)],
                    rhs=LW[:, 0:Q_LEN - k * P],
                    start=(k == 0),
                    stop=(k == NKN - 1),
                )

            # k-waves over m 0-3 (start as en[k] tiles land), then
            # m-chains for m 4-7 interleaved with the first main rounds.
            for k in range(NKN):
                for m in range(4):
                    prefix_mm(m, k)

            # ---- evacuation: PSUM -> SBUF fp16 with the old-memory
            # colsum folded in as a per-partition bias (ACT even m /
            # DVE odd m) ----
            at = [None] * NE

            def evac(m):
                o = sb.tile([P, Q_LEN], f16, tag=f"at{m}", name=f"at{m}")
                if m % 2 == 0:
                    nc.scalar.activation(
                        o[:], psA[m][:],
                        mybir.ActivationFunctionType.Identity,
                        bias=cs[:, m:m + 1],
                    )
                else:
                    nc.vector.tensor_scalar(
                        o[:], psA[m][:], cs[:, m:m + 1], None,
                        mybir.AluOpType.add,
                    )
                at[m] = o

            # ---- phase 2 pieces ----
            # outT'[g, t] = sum_e w2[e, g] * A'.T[e, t]
            psO = {}

            def main_chain_mm(g, e):
                if e == 0:
                    psO[g] = pso_pool.tile([P, Q_LEN], f32, tag="psO", name=f"psO{g}")
                nc.tensor.matmul(
                    psO[g][:], lhsT=w2t[e][:, sl(g)], rhs=at[e][:],
                    start=(e == 0), stop=(e == NE - 1),
                )

            def drain(g):
                o = sb.tile([P, Q_LEN], f16, tag=f"ot{g}", name=f"ot{g}")
                if g % 2 == 0:
                    nc.vector.tensor_copy(o[:], psO[g][:])
                else:
                    nc.scalar.copy(o[:], psO[g][:])
                nc.sync.dma_start(out_t[sl(g), :], o[:])

            for m in range(4):
                evac(m)

            # interleave: finish prefix chains m 4-7 while the e-outer
            # main rounds for g 0,1 start consuming at/w2 tiles.
            for k in range(NKN):
                prefix_mm(4, k)
            for k in range(NKN):
                prefix_mm(5, k)
            evac(4)
            evac(5)
            for e in range(2):
                main_chain_mm(0, e), main_chain_mm(1, e)
            for k in range(NKN):
                prefix_mm(6, k)
            evac(6)
            for e in range(2, 4):
                main_chain_mm(0, e), main_chain_mm(1, e)
            for k in range(NKN):
                prefix_mm(7, k)
            evac(7)
            for e in range(4, NE):
                main_chain_mm(0, e), main_chain_mm(1, e)
            drain(0), drain(1)
            # second e-outer pair
            for e in range(NE):
                main_chain_mm(2, e), main_chain_mm(3, e)
            drain(2), drain(3)
            # g-outer tail: staggered output drain
            for g in range(4, NE):
                for e in range(NE):
                    main_chain_mm(g, e)
                drain(g)

    nc.compile()
    return nc


def _get_program():
    if "nc" not in _PROGRAM_CACHE:
        _PROGRAM_CACHE["nc"] = _build_program()
    return _PROGRAM_CACHE["nc"]


def _make_in_maps(inputs):
    emb_new = np.asarray(inputs["emb_new"], dtype=np.float32)
    emb_old = np.asarray(inputs["emb_old"], dtype=np.float32)
    wkv = np.asarray(inputs["Wkv"], dtype=np.float32)
    wfc = np.asarray(inputs["Wfc"], dtype=np.float32)
    mask = np.asarray(inputs["mask"]).reshape(Q_LEN, KLEN)

    # 1/count row normalization (diagonal on t; commutes to the end).
    nm = ~mask
    inv_count = (1.0 / nm.sum(axis=1)).astype(np.float64)  # [q]

    # Compile-time weight folding: W2 = Wkv_v @ Wfc.
    w2 = (wkv[:, HD:].astype(np.float64) @ wfc.astype(np.float64)).astype(np.float16)

    in_maps = []
    for b in range(N_CORES):
        cs_b = emb_old[:, b, :].sum(axis=0)  # [emb] colsum of old memory
        in_maps.append(
            {
                "emb_new": np.ascontiguousarray(emb_new[:, b, :]).astype(np.float16),
                "cs": np.ascontiguousarray(cs_b.reshape(NE, P).T.astype(np.float32)),
                "w2": w2,
            }
        )
    return in_maps, inv_count


def _run(inputs, trace=False, trace_cores=None):
    from concourse import bass_utils

    nc = _get_program()
    in_maps, inv_count = _make_in_maps(inputs)
    res = bass_utils.run_bass_kernel_spmd(
        nc,
        in_maps,
        core_ids=list(range(N_CORES)),
        trace=trace,
        trace_cores=trace_cores,
    )
    scale = inv_count[:, None].astype(np.float32)  # [q, 1]
    out = np.empty((Q_LEN, BATCH, EMB), dtype=np.float32)
    for b in range(N_CORES):
        out[:, b, :] = res.results[b]["outT"].astype(np.float32).T * scale
    return out, res


def _mask_is_causal(mask):
    qi = np.arange(Q_LEN)[:, None]
    ki = np.arange(KLEN)[None, :]
    return bool(np.array_equal(mask, ki > (qi + MEM_LEN)))


def _host_fallback(inputs, mask):
    """Numpy masked-mean path, used only if the mask is not the standard
    causal-with-memory pattern baked into the device program."""
    emb_new = np.asarray(inputs["emb_new"], dtype=np.float64)
    emb_old = np.asarray(inputs["emb_old"], dtype=np.float64)
    wkv = np.asarray(inputs["Wkv"], dtype=np.float64)
    wfc = np.asarray(inputs["Wfc"], dtype=np.float64)
    nm = (~mask).astype(np.float64)
    m = nm / nm.sum(axis=1, keepdims=True)
    emb_full = np.concatenate([emb_old, emb_new], axis=0)
    x = np.einsum("qk,kbe->qbe", m, emb_full)
    return (x @ wkv[:, HD:] @ wfc).astype(np.float32)


def kernel(**inputs):
    mask = np.asarray(inputs["mask"]).reshape(Q_LEN, KLEN)
    if not _mask_is_causal(mask):
        return _host_fallback(inputs, mask)
    out, _ = _run(inputs)
    return out


# revision 6
# speedup vs baseline: 1.3523x; 1.0103x over previous
"""Trainium2 Bass kernel for nn_MultiHeadAttention_46093589021334.

Transformer-XL style multi-head attention with SCALE = 1/D**5 ~= 9.3e-10
(faithful to the source module). At that scale every attention logit is
O(1e-9) after scaling, so softmax(attn * SCALE) equals the uniform
distribution over unmasked key positions to one part in 1e8 -- far below
fp32 roundoff of the reference itself.  The module output is therefore
(exactly, to fp32 precision):

    out[:, b, :] = (M @ emb_b) @ Wkv[:, H*D:] @ Wfc

where emb_b = concat(emb_old, emb_new)[:, b, :]  (klen x emb) and
M[t, j] = (not mask[t, j]) / (# unmasked j in row t)   (q x klen).

Restructurings on top of the baseline:

1. Weight folding (host, compile-time): W2 = Wkv[:, H*D:] @ Wfc is a
   fixed [emb, emb] matrix -- fold it once on the host.  The device
   chain becomes two matmuls instead of three.

2. Prefix structure of M: row t of the unnormalized not-mask sums ALL
   of emb_old plus a PREFIX of emb_new (j <= t).  On device:
       A'.T[e, t] = colsum(emb_old)[e] + sum_{j<=t} emb_new[j, e]
   The prefix is a matmul against a lower-triangular ones matrix; only
   the 128x128 diagonal block is ever triangular, so a single SBUF
   tile LW = [tri(128) | ones(512)] serves every k-wave as a prefix
   slice LW[:, 0:512-128k] (10240 PE cycles vs 27648 for the dense
   masked matmul).  The colsum term is input prep on the host (4 KB
   bias tensor, same spirit as the baseline's host-side inv_count),
   folded in for free as a per-partition bias during the PSUM->SBUF
   evacuation.  The 1/count(t) normalization is diagonal on t (the
   moving axis of both matmuls), so it commutes to the end and is
   applied on the host during the gather, as in the baseline.

Everything on device runs in fp16 (tolerance 2e-2; fp16 adds ~5e-4):
halves DMA traffic and enables fast weight load on the PE.

Scheduling (v4, from trace analysis):
- All DMAs ride the SP (sync) HWDGE ring in consumption order
  (emb_new, cs, w2, outputs); the ACT engine stays free for PSUM
  evacuations.  DMA completion semaphores land ~2.6us after the data,
  so the PE covers the gap with ~3us of warmup matmuls on a
  gpsimd-memset tile (also flips the HAM clock gate to 8/8 before the
  real work: cold matmuls run at 1.2 GHz, warm at 2.4).
- Main matmul: two e-outer pair-chains (g0/g1, g2/g3) start consuming
  at/w2 tiles as they land, interleaved with the remaining prefix
  chains; their e=7 matmuls are DEFERRED to the end so the last w2
  tile's late semaphore cannot block the in-order PE queue.  g4-g7
  run g-outer for a staggered output drain.
- PSUM pair tiles [128, 1024] (2 banks) hold two g-tiles each, so a
  pair drains with ONE copy + ONE 256 KB DMA.  Output dram tensor is
  [128, 8, 512] (g-tile-major); the host re-permutes in the gather.
- PSUM budget: 4 banks prefix accumulators + 2x2-bank main pairs.

Distribution: data-parallel over batch. BATCH == 8 == n_cores; no
collectives.
"""

import sys

if "/opt/trn_rl_repo" not in sys.path:
    sys.path.insert(0, "/opt/trn_rl_repo")

import numpy as np

P = 128
Q_LEN = 512
MEM_LEN = 512
KLEN = 1024
BATCH = 8
EMB = 1024
HD = 1024  # H * D
N_CORES = 8
NE = EMB // P     # e tiles (8)
NKN = Q_LEN // P  # new-key tiles (4)
N_WARM = 14

_PROGRAM_CACHE = {}


def _build_program():
    """Build + bacc-compile the per-core Bass program (cached)."""
    import concourse.bacc as bacc
    import concourse.mybir as mybir
    import concourse.tile as tile

    nc = bacc.Bacc(
        "TRN2",
        target_bir_lowering=False,
        debug=False,
        enable_asserts=False,
        num_devices=N_CORES,
    )
    f32 = mybir.dt.float32
    f16 = mybir.dt.float16

    emb_new = nc.dram_tensor("emb_new", [Q_LEN, EMB], f16, kind="ExternalInput").ap()
    cs_in = nc.dram_tensor("cs", [P, NE], f32, kind="ExternalInput").ap()
    w2 = nc.dram_tensor("w2", [EMB, EMB], f16, kind="ExternalInput").ap()
    # g-tile-major output: outT2[p, g, t] = outT'[128g + p, t]
    out_t = nc.dram_tensor("outT2", [P, NE, Q_LEN], f16, kind="ExternalOutput").ap()

    with tile.TileContext(nc) as tc:
        with (
            tc.tile_pool(name="sb", bufs=1) as sb,
            tc.tile_pool(name="psA", bufs=4, space="PSUM") as psa_pool,
            tc.tile_pool(name="psO", bufs=2, space="PSUM") as pso_pool,
        ):
            sl = lambda m: slice(m * P, (m + 1) * P)

            # ---- warm tile: all-ones via gpsimd memset (fast, no deps) ----
            wt = sb.tile([P, 256], f16, tag="wt")
            nc.gpsimd.memset(wt[:], 1.0)

            # ---- LW = [tri(128) | ones(512)]: wave k's prefix operand is
            # LW[:, 0:512-128k] ----
            LW = sb.tile([P, P + Q_LEN], f16, tag="LW")
            nc.vector.memset(LW[:], 1.0)
            iota_t = sb.tile([P, P], f32, tag="iota")
            nc.gpsimd.iota(
                iota_t[:], [[1, P]], base=0, channel_multiplier=-1,
                allow_small_or_imprecise_dtypes=True,
            )
            nc.vector.tensor_scalar(
                LW[:, 0:P], iota_t[:], 0.0, None, mybir.AluOpType.is_ge
            )

            # ---- input DMAs, all on the SP (sync) HWDGE ring in
            # consumption order ----
            en = []
            for k in range(NKN):
                t = sb.tile([P, EMB], f16, tag=f"en{k}")
                nc.sync.dma_start(t[:], emb_new[k * P:(k + 1) * P, :])
                en.append(t)
            cs = sb.tile([P, NE], f32, tag="cs")
            nc.sync.dma_start(cs[:], cs_in[:, :])
            w2t = []
            for e in range(NE):
                t = sb.tile([P, EMB], f16, tag=f"w2{e}")
                nc.sync.dma_start(t[:], w2[e * P:(e + 1) * P, :])
                w2t.append(t)

            # ---- PE warmup: bridges the first DMA's completion-semaphore
            # latency and flips HAM to 8/8 ----
            warm = psa_pool.tile([P, 256], f32, tag="psA", name="warm")
            for _ in range(N_WARM):
                nc.tensor.matmul(
                    warm[:], lhsT=wt[:, :P], rhs=wt[:],
                    start=True, stop=True,
                )

            # ---- phase 1: prefix-sum matmuls ----
            # psA[m][ee, t] = sum_{j<=t} emb_new[j, 128m+ee]
            psA = [None] * NE

            def prefix_mm(m, k):
                if k == 0:
                    psA[m] = psa_pool.tile([P, Q_LEN], f32, tag="psA", name=f"psA{m}")
                nc.tensor.matmul(
                    psA[m][:, k * P:],
                    lhsT=en[k][:, sl(m)],
                    rhs=LW[:, 0:Q_LEN - k * P],
                    start=(k == 0),
                    stop=(k == NKN - 1),
                )

            # ---- evacuation: PSUM -> SBUF fp16 with the old-memory colsum
            # folded in as a per-partition bias (ACT even m / DVE odd m) ----
            at = [None] * NE

            def evac(m):
                o = sb.tile([P, Q_LEN], f16, tag=f"at{m}", name=f"at{m}")
                if m % 2 == 0:
                    nc.scalar.activation(
                        o[:], psA[m][:],
                        mybir.ActivationFunctionType.Identity,
                        bias=cs[:, m:m + 1],
                    )
                else:
                    nc.vector.tensor_scalar(
                        o[:], psA[m][:], cs[:, m:m + 1], None,
                        mybir.AluOpType.add,
                    )
                at[m] = o

            # ---- phase 2: outT'[g, t] = sum_e w2[e, g] A'.T[e, t] ----
            # pair tile p holds g = 2p (cols 0:512) and g = 2p+1 (cols 512:).
            po = [None] * 4

            def main_mm(pair, g, e, stop=False):
                if po[pair] is None:
                    po[pair] = pso_pool.tile(
                        [P, 2 * Q_LEN], f32, tag="psO", name=f"po{pair}"
                    )
                half = (g % 2) * Q_LEN
                nc.tensor.matmul(
                    po[pair][:, half:half + Q_LEN],
                    lhsT=w2t[e][:, sl(g)], rhs=at[e][:],
                    start=(e == 0), stop=stop,
                )

            def drain(pair):
                o = sb.tile([P, 2 * Q_LEN], f16, tag=f"ot{pair}", name=f"ot{pair}")
                if pair % 2 == 0:
                    nc.vector.tensor_copy(o[:], po[pair][:])
                else:
                    nc.scalar.copy(o[:], po[pair][:])
                nc.sync.dma_start(
                    out_t[:, 2 * pair:2 * pair + 2, :],
                    o[:].rearrange("p (c f) -> p c f", c=2),
                )
                po[pair] = None

            # -- k-waves over m 0-3: start as en[k] completion sems land --
            for k in range(NKN):
                for m in range(4):
                    prefix_mm(m, k)
            evac(0), evac(1)

            # -- remaining prefix chains interleaved with the first
            # e-outer main rounds (pairs 0 and 1); e=7 deferred --
            for k in range(NKN):
                prefix_mm(4, k)
            evac(2)
            main_mm(0, 0, 0), main_mm(0, 1, 0)
            for k in range(NKN):
                prefix_mm(5, k)
            evac(3), evac(4)
            main_mm(0, 0, 1), main_mm(0, 1, 1)
            for k in range(NKN):
                prefix_mm(6, k)
            evac(5)
            main_mm(0, 0, 2), main_mm(0, 1, 2)
            for k in range(NKN):
                prefix_mm(7, k)
            evac(6), evac(7)
            for e in range(3, 7):
                main_mm(0, 0, e), main_mm(0, 1, e)
            for e in range(7):
                main_mm(1, 2, e), main_mm(1, 3, e)
            # deferred e=7 for pairs 0/1 (last w2 tile's semaphore is late)
            main_mm(0, 0, 7, stop=True), main_mm(0, 1, 7, stop=True)
            main_mm(1, 2, 7, stop=True), main_mm(1, 3, 7, stop=True)
            drain(0)
            drain(1)
            # -- g-outer tail with staggered paired drains --
            for pair in (2, 3):
                for g in (2 * pair, 2 * pair + 1):
                    for e in range(NE):
                        main_mm(pair, g, e, stop=(e == NE - 1))
                drain(pair)

    nc.compile()
    return nc


def _get_program():
    if "nc" not in _PROGRAM_CACHE:
        _PROGRAM_CACHE["nc"] = _build_program()
    return _PROGRAM_CACHE["nc"]


def _make_in_maps(inputs):
    emb_new = np.asarray(inputs["emb_new"], dtype=np.float32)
    emb_old = np.asarray(inputs["emb_old"], dtype=np.float32)
    wkv = np.asarray(inputs["Wkv"], dtype=np.float32)
    wfc = np.asarray(inputs["Wfc"], dtype=np.float32)
    mask = np.asarray(inputs["mask"]).reshape(Q_LEN, KLEN)

    # 1/count row normalization (diagonal on t; commutes to the end).
    nm = ~mask
    inv_count = (1.0 / nm.sum(axis=1)).astype(np.float64)  # [q]

    # Compile-time weight folding: W2 = Wkv_v @ Wfc.
    w2 = (wkv[:, HD:].astype(np.float64) @ wfc.astype(np.float64)).astype(np.float16)

    in_maps = []
    for b in range(N_CORES):
        cs_b = emb_old[:, b, :].sum(axis=0)  # [emb] colsum of old memory
        in_maps.append(
            {
                "emb_new": np.ascontiguousarray(emb_new[:, b, :]).astype(np.float16),
                "cs": np.ascontiguousarray(cs_b.reshape(NE, P).T.astype(np.float32)),
                "w2": w2,
            }
        )
    return in_maps, inv_count


def _run(inputs, trace=False, trace_cores=None):
    from concourse import bass_utils

    nc = _get_program()
    in_maps, inv_count = _make_in_maps(inputs)
    res = bass_utils.run_bass_kernel_spmd(
        nc,
        in_maps,
        core_ids=list(range(N_CORES)),
        trace=trace,
        trace_cores=trace_cores,
    )
    scale = inv_count[:, None].astype(np.float32)  # [q, 1]
    out = np.empty((Q_LEN, BATCH, EMB), dtype=np.float32)
    for b in range(N_CORES):
        o = res.results[b]["outT2"].astype(np.float32)  # [128, 8, 512]
        outT = o.transpose(1, 0, 2).reshape(EMB, Q_LEN)
        out[:, b, :] = outT.T * scale
    return out, res


def _mask_is_causal(mask):
    qi = np.arange(Q_LEN)[:, None]
    ki = np.arange(KLEN)[None, :]
    return bool(np.array_equal(mask, ki > (qi + MEM_LEN)))


def _host_fallback(inputs, mask):
    """Numpy masked-mean path, used only if the mask is not the standard
    causal-with-memory pattern baked into the device program."""
    emb_new = np.asarray(inputs["emb_new"], dtype=np.float64)
    emb_old = np.asarray(inputs["emb_old"], dtype=np.float64)
    wkv = np.asarray(inputs["Wkv"], dtype=np.float64)
    wfc = np.asarray(inputs["Wfc"], dtype=np.float64)
    nm = (~mask).astype(np.float64)
    m = nm / nm.sum(axis=1, keepdims=True)
    emb_full = np.concatenate([emb_old, emb_new], axis=0)
    x = np.einsum("qk,kbe->qbe", m, emb_full)
    return (x @ wkv[:, HD:] @ wfc).astype(np.float32)


def kernel(**inputs):
    mask = np.asarray(inputs["mask"]).reshape(Q_LEN, KLEN)
    if not _mask_is_causal(mask):
        return _host_fallback(inputs, mask)
    out, _ = _run(inputs)
    return out
